# revision 19
# baseline (speedup 1.0000x reference)
"""Trainium2 Bass kernel for HF-style causal self-attention (B=2, S=2048, D=2048,
H=16, head_dim=128), tensor-parallel over heads across 8 NeuronCores.

Sharding: core c computes heads {2c, 2c+1} for both batches (column-sharded
Wq/Wk/Wv). After per-head attention, an 8-rank AllToAll redistributes the
per-head context from head-sharding to token-sharding, and each core runs the
output projection (full Wo) for its 512-token slice. The host concatenates the
8 token slices.

Per-core dataflow (all matmuls run as float32r = full-fp32 at 1 cycle/row):
  QK phase : Q^T,K^T [hd=128, 4096 tok] = Wq/Wk-slice.T @ x, RoPE applied via a
             128x128 rotation matmul + DVE combine with cos/sin tables.
  V phase  : V [tok, 256] natural layout (lhsT = x^T tile).
  Attention: per (head, batch, 512-query-group): scores^T [k,q] chunks on PE,
             causal-trimmed by a runtime classification of the mask;
             exp on ACT (scale=1/sqrt(hd), no max-subtraction -- inputs are
             unit-variance so scores are O(5)); row sums via an all-ones
             matmul accumulated alongside P@V; normalize during PSUM
             evacuation with DVE reciprocal+multiply.
  AllToAll : [8, 256, 512] shards (token groups) -> full-feature 512-token slice.
  O proj   : out[512, 2048] = attn.T @ Wo, streamed Wo tiles.
"""

import math
import os

import numpy as np

# ---------------------------------------------------------------- constants
B, S, D = 2, 2048, 2048
H, HD = 16, 128
N_CORES = 8
LOCAL_H = H // N_CORES  # 2 heads per core
LOCAL_F = LOCAL_H * HD  # 256 local features
TOKS = B * S  # 4096 flat tokens (batch-major)
TG = 512  # token-group width (matmul moving dim)
NT = TOKS // TG  # 8 token groups
NB = S // 128  # 16 key blocks per batch
QB = TG // 128  # 4 query blocks per group
ROPE_BASE = 10000.0
SCALE = 1.0 / math.sqrt(HD)
SKIP_THRESH = -1e8  # mask block entirely masked if all values below this

_NC_CACHE: dict = {}
last_exec_time_ns = None


# ---------------------------------------------------------------- host prep
def _rope_tables():
    inv_freq = 1.0 / (ROPE_BASE ** (np.arange(0, HD, 2, dtype=np.float64) / HD))
    t = np.arange(S, dtype=np.float64)
    freqs = np.outer(t, inv_freq)  # [S, HD/2]
    emb = np.concatenate([freqs, freqs], axis=-1)  # [S, HD]
    cos = np.cos(emb).astype(np.float32).T  # [HD, S]
    sin = np.sin(emb).astype(np.float32).T
    return np.ascontiguousarray(cos), np.ascontiguousarray(sin)


def _rot_matrix():
    # rot = Rt.T @ q  where rot[d] = -q[d+64] (d<64), rot[d] = q[d-64] (d>=64)
    rt = np.zeros((HD, HD), dtype=np.float32)
    half = HD // 2
    for m in range(half):
        rt[m + half, m] = -1.0
    for m in range(half, HD):
        rt[m - half, m] = 1.0
    return rt


def _classify_mask(mask2d):
    """Per 128x128 block of mask[q, k]: 0=all-zero, 1=needs add, 2=fully masked."""
    nq, nk = S // 128, S // 128
    blocks = mask2d.reshape(nq, 128, nk, 128)
    mx = blocks.max(axis=(1, 3))
    mn = blocks.min(axis=(1, 3))
    cls = np.ones((nq, nk), dtype=np.int8)
    cls[(mx == 0.0) & (mn == 0.0)] = 0
    cls[mx < SKIP_THRESH] = 2
    return cls


def _build_plan(cls):
    """For each (query group g, key block j): None if skipped, else
    (comp_start_lqb, add_start_lqb, add_nblocks). The add range spans the
    first to last local query block needing a mask add (interior all-zero
    blocks contribute zeros, so one combined add is exact)."""
    plan = {}
    for g in range(4):
        for j in range(NB):
            lcls = [cls[4 * g + l, j] for l in range(QB)]
            if all(c == 2 for c in lcls):
                continue
            comp = min(l for l in range(QB) if lcls[l] != 2)
            comp = min(comp, 2)  # keep moving dim >= 256 for float32r speed
            adds = [l for l in range(comp, QB) if lcls[l] != 0]
            if adds:
                plan[(g, j)] = (comp, adds[0], adds[-1] - adds[0] + 1)
            else:
                plan[(g, j)] = (comp, 0, 0)
    return plan


def _plan_key(plan):
    return tuple(sorted((k, v) for k, v in plan.items()))


# ---------------------------------------------------------------- bass build
def _build(plan, n_mask_blocks, mask_idx, prec="f32r"):
    import concourse.bacc as bacc
    import concourse.mybir as mybir
    import concourse.tile as tile

    f32 = mybir.dt.float32
    # matmul operand dtype: float32r streams 1 col/cycle (vs 4 for float32)
    # at the cost of ~2^-13 input rounding; toggled for accuracy fallback.
    mm = mybir.dt.float32r if prec == "f32r" else f32
    Exp = mybir.ActivationFunctionType.Exp

    nc = bacc.Bacc("TRN2", target_bir_lowering=False, debug=False,
                   num_devices=N_CORES)

    xT = nc.dram_tensor("xT", [D, TOKS], mm, kind="ExternalInput").ap()
    wq = nc.dram_tensor("wq", [D, LOCAL_F], mm, kind="ExternalInput").ap()
    wk = nc.dram_tensor("wk", [D, LOCAL_F], mm, kind="ExternalInput").ap()
    wv = nc.dram_tensor("wv", [D, LOCAL_F], mm, kind="ExternalInput").ap()
    wo = nc.dram_tensor("wo", [D, D], mm, kind="ExternalInput").ap()
    maskc = nc.dram_tensor("maskc", [max(n_mask_blocks, 1), 128, 512], f32,
                           kind="ExternalInput").ap()
    cosT = nc.dram_tensor("cosT", [HD, S], f32, kind="ExternalInput").ap()
    sinT = nc.dram_tensor("sinT", [HD, S], f32, kind="ExternalInput").ap()
    rtm = nc.dram_tensor("rtm", [HD, HD], mm, kind="ExternalInput").ap()
    onesd = nc.dram_tensor("onesd", [128, 128], mm, kind="ExternalInput").ap()
    out = nc.dram_tensor("out", [TG, D], f32, kind="ExternalOutput").ap()

    with tile.TileContext(nc) as tc:
        with (
            tc.tile_pool(name="const", bufs=1) as constp,
            tc.tile_pool(name="dram", bufs=1, space="DRAM") as dram,
        ):
            # ---- constants
            cos_t = constp.tile([HD, S], f32, tag="cos")
            sin_t = constp.tile([HD, S], f32, tag="sin")
            rt_t = constp.tile([HD, HD], mm, tag="rt")
            ones_t = constp.tile([128, 128], mm, tag="ones")

            _attention_body(nc, tc, tile, mybir, mm, plan, mask_idx,
                            cos_t, sin_t, rt_t, ones_t,
                            xT, wq, wk, wv, wo, out, dram,
                            maskc, cosT, sinT, rtm, onesd)

    nc.compile()
    return nc


def _attention_body(nc, tc, tile, mybir, mm, plan, mask_idx,
                    cos_t, sin_t, rt_t, ones_t,
                    xT, wq, wk, wv, wo, out, dram,
                    maskc, cosT, sinT, rtm, onesd):
    f32 = mybir.dt.float32
    Exp = mybir.ActivationFunctionType.Exp
    ND = D // 128  # 16 contraction chunks

    qkv_scope = tc.tile_pool(name="qkres", bufs=LOCAL_H)
    vres_scope = tc.tile_pool(name="vres", bufs=4 * NT)
    qkres = qkv_scope.__enter__()
    vres = vres_scope.__enter__()

    # resident Q^T / K^T per local head [128, TOKS]
    qt = [qkres.tile([HD, TOKS], mm, tag="qt", name=f"qt{i}") for i in range(LOCAL_H)]
    kt = [qkres.tile([HD, TOKS], mm, tag="kt", name=f"kt{i}") for i in range(LOCAL_H)]
    # resident V tiles [128 tok, LOCAL_F] per flat token block
    v_sb = [vres.tile([128, LOCAL_F], mm, tag="v", name=f"v{i}") for i in range(TOKS // 128)]

    # ---------------- phase 1: QKV projections in ONE x^T sweep
    # Per token group: 8 PSUM accumulators (Q x2 heads, K x2 heads, V x4
    # token blocks); each x^T tile feeds 8 matmuls then releases. RoPE
    # rotation matmuls reuse accumulator slots freed by the Q/K evacuation.
    with (
        tc.tile_pool(name="wpool", bufs=3 * ND) as wpool,
        tc.tile_pool(name="xpool", bufs=8) as xpool,
        tc.tile_pool(name="ropes", bufs=3) as ropes,
        tc.tile_pool(name="ropet", bufs=2) as ropet1,
        tc.tile_pool(name="ropeu", bufs=2) as ropet2,
        tc.tile_pool(name="psqk", bufs=8, space="PSUM") as psqk,
    ):
        wq_t = [wpool.tile([128, LOCAL_F], mm, tag="w", name=f"wqt{i}") for i in range(ND)]
        wk_t = [wpool.tile([128, LOCAL_F], mm, tag="w", name=f"wkt{i}") for i in range(ND)]
        wv_t = [wpool.tile([128, LOCAL_F], mm, tag="w", name=f"wvt{i}") for i in range(ND)]

        for t in range(NT):
            tsl = slice(TG * t, TG * (t + 1))
            csl = slice(TG * (t % 4), TG * (t % 4 + 1))  # batch-local cols
            acc = [psqk.tile([HD, TG], f32, tag="qk", name=f"acc{i}")
                   for i in range(2 * LOCAL_H)]
            vacc = [psqk.tile([128, TG], f32, tag="qk", name=f"vacc{i}")
                    for i in range(4)]
            for j in range(ND):
                if t == 0:
                    nc.sync.dma_start(out=wq_t[j], in_=wq[128 * j:128 * (j + 1), :])
                    nc.sync.dma_start(out=wk_t[j], in_=wk[128 * j:128 * (j + 1), :])
                    nc.sync.dma_start(out=wv_t[j], in_=wv[128 * j:128 * (j + 1), :])
                if t == 0 and j == 2:
                    # constants needed from the first RoPE evac onward; emitted
                    # here so they don't delay the first matmul's operands
                    nc.sync.dma_start(out=cos_t, in_=cosT)
                    nc.sync.dma_start(out=sin_t, in_=sinT)
                    nc.sync.dma_start(out=rt_t, in_=rtm)
                    nc.sync.dma_start(out=ones_t, in_=onesd)
                x_t = xpool.tile([128, TG], mm, tag="x")
                nc.sync.dma_start(out=x_t, in_=xT[128 * j:128 * (j + 1), tsl])
                for wi, w_t in enumerate((wq_t, wk_t)):
                    for h in range(LOCAL_H):
                        hsl = slice(128 * h, 128 * (h + 1))
                        nc.tensor.matmul(acc[2 * wi + h], w_t[j][:, hsl],
                                         x_t,
                                         start=(j == 0), stop=(j == ND - 1))
                for m in range(4):
                    msl = slice(128 * m, 128 * (m + 1))
                    nc.tensor.matmul(vacc[m][:, :LOCAL_F], x_t[:, msl],
                                     wv_t[j],
                                     start=(j == 0), stop=(j == ND - 1))
            for wi, res in ((0, qt), (1, kt)):
                for h in range(LOCAL_H):
                    ps = acc[2 * wi + h]
                    # RoPE: res = ps * cos + (Rt.T @ ps) * sin
                    s_t = ropes.tile([HD, TG], mm, tag="s")
                    nc.scalar.copy(s_t, ps)
                    rp = psqk.tile([HD, TG], f32, tag="qk", name="rotp")
                    nc.tensor.matmul(rp, rt_t, s_t, start=True, stop=True)
                    t1 = ropet1.tile([HD, TG], f32, tag="t1")
                    nc.vector.tensor_mul(t1, ps, cos_t[:, csl])
                    t2 = ropet2.tile([HD, TG], f32, tag="t2")
                    nc.vector.tensor_mul(t2, rp, sin_t[:, csl])
                    nc.vector.tensor_add(res[h][:, tsl], t1, t2)
            for m in range(4):
                nc.scalar.copy(v_sb[4 * t + m], vacc[m][:, :LOCAL_F])

    # ---------------- phase 3: attention per (head, batch, query group)
    inb = [dram.tile([N_CORES, HD, TG], mm, name=f"inb{i}")
           for i in range(LOCAL_H)]
    outb = [dram.tile([N_CORES, HD, TG], mm, name=f"outb{i}")
            for i in range(LOCAL_H)]

    wop_scope = tc.tile_pool(name="wop", bufs=10)
    wop = wop_scope.__enter__()
    wo_t = {}

    def load_wo(n, f):
        # per-pass tags: pass1 consumes even f (head-0 features), pass2 odd f;
        # separate slot sets so a pass never holds the other pass's tiles
        tag = "woe" if f % 2 == 0 else "woo"
        w_t = wop.tile([128, TG], mm, tag=tag, name=f"wo{n}_{f}")
        # gpsimd (SWDGE) queue keeps these 16.8MB off the sync queue that
        # carries the latency-critical attnT bounce writes
        nc.gpsimd.dma_start(out=w_t,
                            in_=wo[128 * f:128 * (f + 1),
                                   TG * n:TG * (n + 1)])
        wo_t[(n, f)] = w_t

    # prefetch only as many tiles as the pool has slots; the rest are
    # emitted inside the o_proj loop AFTER the collective triggers, so the
    # in-order gpsimd queue can't cycle (slot release needs o_proj, which
    # needs the AllToAll)
    for n in range(2):
        for f in range(0, ND, 2):
            load_wo(n, f)
    for f in range(1, ND, 2):
        load_wo(0, f)

    with (
        tc.tile_pool(name="maskp", bufs=1) as maskp,
        tc.tile_pool(name="probs", bufs=4) as probs,
        tc.tile_pool(name="recipp", bufs=2) as recipp,
        tc.tile_pool(name="attnp", bufs=4) as attnp,
        tc.tile_pool(name="pssc", bufs=2, space="PSUM") as pssc,
        tc.tile_pool(name="psacc", bufs=2, space="PSUM") as psacc,
    ):
        mask_tiles = {}
        for key, (idx, nb) in mask_idx.items():
            mt = maskp.tile([128, 128 * nb], f32, tag=f"mb{idx}",
                            name=f"mb{idx}")
            nc.sync.dma_start(out=mt, in_=maskc[idx][:, :128 * nb])
            mask_tiles[key] = mt
        # Chunks are processed in pairs sharing one 2-bank PSUM tile and ONE
        # exp (ACT per-op overhead dominates small activations). One pair of
        # lookahead keeps PE (in-order) from stalling on exp latency.
        for h in range(LOCAL_H):
            for b, g in [(b, g) for b in range(B) for g in range(4)]:
                if True:
                    chunks = [(j, plan[(g, j)]) for j in range(NB)
                              if (g, j) in plan]
                    pairs = [chunks[i:i + 2] for i in range(0, len(chunks), 2)]
                    qsl = slice(2048 * b + TG * g, 2048 * b + TG * (g + 1))
                    sum_ps = psacc.tile([128, TG], f32, tag="sum")
                    pv_ps = psacc.tile([HD, TG], f32, tag="pv")
                    first = chunks[0][0]
                    last = chunks[-1][0]

                    def emit_pair(pr, h=h, b=b, g=g, qsl=qsl):
                        sc = pssc.tile([128, 2 * TG], f32, tag="sc",
                                       name="sc")
                        pt = probs.tile([128, 2 * TG], mm, tag="p",
                                        name="pt")
                        out = []
                        for half, (j, (comp, a0, nb)) in enumerate(pr):
                            co = 128 * comp
                            off = TG * half
                            ksl = slice(2048 * b + 128 * j,
                                        2048 * b + 128 * (j + 1))
                            nc.tensor.matmul(sc[:, off + co:off + TG],
                                             kt[h][:, ksl],
                                             qt[h][:, qsl][:, co:],
                                             start=True, stop=True)
                            if nb:
                                mt = mask_tiles[(g, j)]
                                q0 = off + 128 * a0
                                nc.vector.tensor_add(
                                    sc[:, q0:q0 + 128 * nb],
                                    sc[:, q0:q0 + 128 * nb], mt)
                            out.append((j, co, off))
                        co0 = out[0][1]
                        end = out[-1][2] + TG
                        nc.scalar.activation(pt[:, co0:end], sc[:, co0:end],
                                             Exp, scale=SCALE)
                        return [(j, co, off, pt) for j, co, off in out]

                    staged = [emit_pair(pairs[0])]
                    for pi in range(len(pairs)):
                        if pi + 1 < len(pairs):
                            staged.append(emit_pair(pairs[pi + 1]))
                        for j, co, off, pt in staged.pop(0):
                            nc.tensor.matmul(
                                sum_ps[:, co:], ones_t,
                                pt[:, off + co:off + TG],
                                start=(j == first), stop=(j == last))
                            kb = 16 * b + j
                            nc.tensor.matmul(
                                pv_ps[:, co:],
                                v_sb[kb][:, 128 * h:128 * (h + 1)],
                                pt[:, off + co:off + TG],
                                start=(j == first), stop=(j == last))
                    rec = recipp.tile([128, TG], f32, tag="rec")
                    nc.vector.reciprocal(rec, sum_ps)
                    at = attnp.tile([HD, TG], mm, tag="at")
                    nc.vector.tensor_mul(at, pv_ps, rec)
                    s = 4 * b + g  # flat token group = destination rank
                    nc.sync.dma_start(out=inb[h][s], in_=at)
            # AllToAll for this head (head-sharded -> token-sharded);
            # h=0's collective overlaps h=1's attention compute
            nc.gpsimd.collective_compute(
                "AllToAll", mybir.AluOpType.bypass,
                replica_groups=[list(range(N_CORES))],
                ins=[inb[h].opt()], outs=[outb[h].opt()],
            )


    # ---------------- phase 5: output projection for my 512-token slice
    with (
        tc.tile_pool(name="afull", bufs=D // 128) as afull,
        tc.tile_pool(name="outp", bufs=3) as outp,
        tc.tile_pool(name="psop", bufs=3, space="PSUM") as psop,
    ):
        af = [None] * (D // 128)
        for f in ([f for f in range(D // 128) if f % LOCAL_H == 0]
                  + [f for f in range(D // 128) if f % LOCAL_H != 0]):
            a_t = afull.tile([128, TG], mm, tag="af", name=f"af{f}")
            nc.sync.dma_start(out=a_t, in_=outb[f % LOCAL_H][f // LOCAL_H])
            af[f] = a_t
        # pass 1: head-0 feature chunks only -- these land with the first
        # AllToAll, so this entire pass overlaps the second collective.
        # Partial sums are stashed in the dead qt tiles (attention is done
        # with them by now).
        evens = [f for f in range(ND) if f % LOCAL_H == 0]
        odds = [f for f in range(ND) if f % LOCAL_H != 0]
        for n in range(4):
            for f in evens:
                if (n, f) not in wo_t:
                    load_wo(n, f)
            for m in range(4):
                p = 4 * n + m
                ps = psop.tile([128, TG], f32, tag="op", name="op1")
                for i, f in enumerate(evens):
                    nc.tensor.matmul(ps, af[f][:, 128 * m:128 * (m + 1)],
                                     wo_t[(n, f)],
                                     start=(i == 0), stop=(i == len(evens) - 1))
                nc.scalar.copy(qt[p // 8][:, TG * (p % 8):TG * (p % 8 + 1)],
                               ps)
        # pass 2: head-1 feature chunks + the stashed partial
        for n in range(4):
            nsl = slice(TG * n, TG * (n + 1))
            for f in odds:
                load_wo(n, f)
            for m in range(4):
                p = 4 * n + m
                ps = psop.tile([128, TG], f32, tag="op", name="op2")
                for i, f in enumerate(odds):
                    nc.tensor.matmul(ps, af[f][:, 128 * m:128 * (m + 1)],
                                     wo_t[(n, f)],
                                     start=(i == 0), stop=(i == len(odds) - 1))
                o_t = outp.tile([128, TG], f32, tag="o")
                nc.vector.tensor_add(
                    o_t, ps,
                    qt[p // 8][:, TG * (p % 8):TG * (p % 8 + 1)])
                nc.sync.dma_start(out=out[128 * m:128 * (m + 1), nsl], in_=o_t)
    wop_scope.__exit__(None, None, None)
    vres_scope.__exit__(None, None, None)
    qkv_scope.__exit__(None, None, None)


# ---------------------------------------------------------------- entry point
def kernel(x, mask, Wq, Wk, Wv, Wo):
    global last_exec_time_ns
    from concourse.bass_utils import run_bass_kernel_spmd

    x = np.asarray(x, dtype=np.float32)
    mask2d = np.ascontiguousarray(np.asarray(mask, dtype=np.float32)[0, 0])
    Wq = np.asarray(Wq, dtype=np.float32)
    Wk = np.asarray(Wk, dtype=np.float32)
    Wv = np.asarray(Wv, dtype=np.float32)
    Wo = np.ascontiguousarray(np.asarray(Wo, dtype=np.float32))

    # ---- host-side prep
    cls = _classify_mask(mask2d)
    plan = _build_plan(cls)
    maskT_s = None
    mask_idx = {}
    strips = []
    for (g, j), (comp, a0, nb) in sorted(plan.items()):
        if nb == 0:
            continue
        if maskT_s is None:
            maskT_s = np.ascontiguousarray(mask2d.T) * math.sqrt(HD)
        q0 = 512 * g + 128 * a0
        strip = np.zeros((128, 512), dtype=np.float32)
        strip[:, :128 * nb] = maskT_s[128 * j:128 * (j + 1),
                                      q0:q0 + 128 * nb]
        strips.append(strip)
        mask_idx[(g, j)] = (len(strips) - 1, nb)
    maskc = (np.stack(strips) if strips
             else np.zeros((1, 128, 512), dtype=np.float32))

    xTf = np.ascontiguousarray(x.reshape(TOKS, D).T)
    cosT, sinT = _rope_tables()
    rtm = _rot_matrix()

    prec = os.environ.get("KERNEL_PREC", "f32r")
    key = (_plan_key(plan), prec)
    if key not in _NC_CACHE:
        _NC_CACHE[key] = _build(plan, len(strips), mask_idx, prec)
    nc = _NC_CACHE[key]
    ones = np.ones((128, 128), dtype=np.float32)

    in_maps = []
    for c in range(N_CORES):
        fsl = slice(LOCAL_F * c, LOCAL_F * (c + 1))
        in_maps.append({
            "xT": xTf,
            "wq": np.ascontiguousarray(Wq[:, fsl]),
            "wk": np.ascontiguousarray(Wk[:, fsl]),
            "wv": np.ascontiguousarray(Wv[:, fsl]),
            "wo": Wo,
            "maskc": maskc,
            "cosT": cosT,
            "sinT": sinT,
            "rtm": rtm,
            "onesd": ones,
        })

    trace = bool(os.environ.get("KERNEL_TRACE"))
    err = None
    for _ in range(3):
        try:
            res = run_bass_kernel_spmd(nc, in_maps,
                                       core_ids=list(range(N_CORES)),
                                       trace=trace)
            break
        except Exception as e:  # axon transport can be flaky; retry
            err = e
    else:
        raise err

    last_exec_time_ns = res.exec_time_ns
    out_flat = np.concatenate([res.results[c]["out"] for c in range(N_CORES)],
                              axis=0)
    return out_flat.reshape(B, S, D)


# revision 20
# speedup vs baseline: 1.2063x; 1.2063x over previous
"""Trainium2 Bass kernel for HF-style causal self-attention (B=2, S=2048, D=2048,
H=16, head_dim=128), tensor-parallel over heads across 8 NeuronCores.

Sharding: core c computes heads {2c, 2c+1} for both batches (column-sharded
Wq/Wk/Wv). After per-head attention, an 8-rank AllToAll redistributes the
per-head context from head-sharding to token-sharding, and each core runs the
output projection (full Wo) for its 512-token slice. The host concatenates the
8 token slices.

Per-core dataflow (all matmuls run as float32r = full-fp32 at 1 cycle/row):
  QK phase : Q^T,K^T [hd=128, 4096 tok] = Wq/Wk-slice.T @ x, RoPE applied via a
             128x128 rotation matmul + DVE combine with cos/sin tables.
  V phase  : V [tok, 256] natural layout (lhsT = x^T tile).
  Attention: per (head, batch, 512-query-group): scores^T [k,q] chunks on PE,
             causal-trimmed by a runtime classification of the mask;
             exp on ACT (scale=1/sqrt(hd), no max-subtraction -- inputs are
             unit-variance so scores are O(5)); row sums via an all-ones
             matmul accumulated alongside P@V; normalize during PSUM
             evacuation with DVE reciprocal+multiply.
  AllToAll : [8, 256, 512] shards (token groups) -> full-feature 512-token slice.
  O proj   : out[512, 2048] = attn.T @ Wo, streamed Wo tiles.
"""

import math
import os

import numpy as np

# ---------------------------------------------------------------- constants
B, S, D = 2, 2048, 2048
H, HD = 16, 128
N_CORES = 8
LOCAL_H = H // N_CORES  # 2 heads per core
LOCAL_F = LOCAL_H * HD  # 256 local features
TOKS = B * S  # 4096 flat tokens (batch-major)
TG = 512  # token-group width (matmul moving dim)
NT = TOKS // TG  # 8 token groups
NB = S // 128  # 16 key blocks per batch
QB = TG // 128  # 4 query blocks per group
ROPE_BASE = 10000.0
SCALE = 1.0 / math.sqrt(HD)
SKIP_THRESH = -1e8  # mask block entirely masked if all values below this

_NC_CACHE: dict = {}
last_exec_time_ns = None


# ---------------------------------------------------------------- host prep
def _rope_tables():
    inv_freq = 1.0 / (ROPE_BASE ** (np.arange(0, HD, 2, dtype=np.float64) / HD))
    t = np.arange(S, dtype=np.float64)
    freqs = np.outer(t, inv_freq)  # [S, HD/2]
    emb = np.concatenate([freqs, freqs], axis=-1)  # [S, HD]
    cos = np.cos(emb).astype(np.float32).T  # [HD, S]
    sin = np.sin(emb).astype(np.float32).T
    return np.ascontiguousarray(cos), np.ascontiguousarray(sin)


def _rot_matrix():
    # rot = Rt.T @ q  where rot[d] = -q[d+64] (d<64), rot[d] = q[d-64] (d>=64)
    rt = np.zeros((HD, HD), dtype=np.float32)
    half = HD // 2
    for m in range(half):
        rt[m + half, m] = -1.0
    for m in range(half, HD):
        rt[m - half, m] = 1.0
    return rt


def _classify_mask(mask2d):
    """Per 128x128 block of mask[q, k]: 0=all-zero, 1=needs add, 2=fully masked."""
    nq, nk = S // 128, S // 128
    blocks = mask2d.reshape(nq, 128, nk, 128)
    mx = blocks.max(axis=(1, 3))
    mn = blocks.min(axis=(1, 3))
    cls = np.ones((nq, nk), dtype=np.int8)
    cls[(mx == 0.0) & (mn == 0.0)] = 0
    cls[mx < SKIP_THRESH] = 2
    return cls


def _build_plan(cls):
    """For each (query group g, key block j): None if skipped, else
    (comp_start_lqb, add_start_lqb, add_nblocks). The add range spans the
    first to last local query block needing a mask add (interior all-zero
    blocks contribute zeros, so one combined add is exact)."""
    plan = {}
    for g in range(4):
        for j in range(NB):
            lcls = [cls[4 * g + l, j] for l in range(QB)]
            if all(c == 2 for c in lcls):
                continue
            comp = min(l for l in range(QB) if lcls[l] != 2)
            comp = min(comp, 2)  # keep moving dim >= 256 for float32r speed
            adds = [l for l in range(comp, QB) if lcls[l] != 0]
            if adds:
                plan[(g, j)] = (comp, adds[0], adds[-1] - adds[0] + 1)
            else:
                plan[(g, j)] = (comp, 0, 0)
    return plan


def _plan_key(plan):
    return tuple(sorted((k, v) for k, v in plan.items()))


# ---------------------------------------------------------------- bass build
def _build(plan, n_mask_blocks, mask_idx, prec="f32r"):
    import concourse.bacc as bacc
    import concourse.mybir as mybir
    import concourse.tile as tile

    f32 = mybir.dt.float32
    # matmul operand dtype: float32r streams 1 col/cycle (vs 4 for float32)
    # at the cost of ~2^-13 input rounding; toggled for accuracy fallback.
    mm = mybir.dt.float32r if prec == "f32r" else f32
    Exp = mybir.ActivationFunctionType.Exp

    nc = bacc.Bacc("TRN2", target_bir_lowering=False, debug=False,
                   num_devices=N_CORES)

    xT = nc.dram_tensor("xT", [D, TOKS], mm, kind="ExternalInput").ap()
    wq = nc.dram_tensor("wq", [D, LOCAL_F], mm, kind="ExternalInput").ap()
    wk = nc.dram_tensor("wk", [D, LOCAL_F], mm, kind="ExternalInput").ap()
    wv = nc.dram_tensor("wv", [D, LOCAL_F], mm, kind="ExternalInput").ap()
    wo = nc.dram_tensor("wo", [D, D], mm, kind="ExternalInput").ap()
    maskc = nc.dram_tensor("maskc", [max(n_mask_blocks, 1), 128, 512], f32,
                           kind="ExternalInput").ap()
    cosT = nc.dram_tensor("cosT", [HD, S], f32, kind="ExternalInput").ap()
    sinT = nc.dram_tensor("sinT", [HD, S], f32, kind="ExternalInput").ap()
    rtm = nc.dram_tensor("rtm", [HD, HD], mm, kind="ExternalInput").ap()
    onesd = nc.dram_tensor("onesd", [128, 128], mm, kind="ExternalInput").ap()
    out = nc.dram_tensor("out", [TG, D], f32, kind="ExternalOutput").ap()

    with tile.TileContext(nc) as tc:
        with (
            tc.tile_pool(name="const", bufs=1) as constp,
            tc.tile_pool(name="dram", bufs=1, space="DRAM") as dram,
        ):
            # ---- constants
            cos_t = constp.tile([HD, S], f32, tag="cos")
            sin_t = constp.tile([HD, S], f32, tag="sin")
            rt_t = constp.tile([HD, HD], mm, tag="rt")
            ones_t = constp.tile([128, 128], mm, tag="ones")

            _attention_body(nc, tc, tile, mybir, mm, plan, mask_idx,
                            cos_t, sin_t, rt_t, ones_t,
                            xT, wq, wk, wv, wo, out, dram,
                            maskc, cosT, sinT, rtm, onesd)

    nc.compile()
    return nc


def _attention_body(nc, tc, tile, mybir, mm, plan, mask_idx,
                    cos_t, sin_t, rt_t, ones_t,
                    xT, wq, wk, wv, wo, out, dram,
                    maskc, cosT, sinT, rtm, onesd):
    f32 = mybir.dt.float32
    Exp = mybir.ActivationFunctionType.Exp
    ND = D // 128  # 16 contraction chunks

    qkv_scope = tc.tile_pool(name="qkres", bufs=LOCAL_H)
    vres_scope = tc.tile_pool(name="vres", bufs=4 * NT)
    qkres = qkv_scope.__enter__()
    vres = vres_scope.__enter__()

    # resident Q^T / K^T per local head [128, TOKS]
    qt = [qkres.tile([HD, TOKS], mm, tag="qt", name=f"qt{i}") for i in range(LOCAL_H)]
    kt = [qkres.tile([HD, TOKS], mm, tag="kt", name=f"kt{i}") for i in range(LOCAL_H)]
    # resident V tiles [128 tok, LOCAL_F] per flat token block
    v_sb = [vres.tile([128, LOCAL_F], mm, tag="v", name=f"v{i}") for i in range(TOKS // 128)]

    # ---------------- phase 1: QKV projections in ONE x^T sweep
    # Per token group: 8 PSUM accumulators (Q x2 heads, K x2 heads, V x4
    # token blocks); each x^T tile feeds 8 matmuls then releases. RoPE
    # rotation matmuls reuse accumulator slots freed by the Q/K evacuation.
    with (
        tc.tile_pool(name="wpool", bufs=3 * ND) as wpool,
        tc.tile_pool(name="xpool", bufs=8) as xpool,
        tc.tile_pool(name="ropes", bufs=3) as ropes,
        tc.tile_pool(name="ropet", bufs=2) as ropet1,
        tc.tile_pool(name="ropeu", bufs=2) as ropet2,
        tc.tile_pool(name="psqk", bufs=8, space="PSUM") as psqk,
    ):
        wq_t = [wpool.tile([128, LOCAL_F], mm, tag="w", name=f"wqt{i}") for i in range(ND)]
        wk_t = [wpool.tile([128, LOCAL_F], mm, tag="w", name=f"wkt{i}") for i in range(ND)]
        wv_t = [wpool.tile([128, LOCAL_F], mm, tag="w", name=f"wvt{i}") for i in range(ND)]

        for t in range(NT):
            tsl = slice(TG * t, TG * (t + 1))
            csl = slice(TG * (t % 4), TG * (t % 4 + 1))  # batch-local cols
            acc = [psqk.tile([HD, TG], f32, tag="qk", name=f"acc{i}")
                   for i in range(2 * LOCAL_H)]
            vacc = [psqk.tile([128, TG], f32, tag="qk", name=f"vacc{i}")
                    for i in range(4)]
            for j in range(ND):
                if t == 0:
                    nc.sync.dma_start(out=wq_t[j], in_=wq[128 * j:128 * (j + 1), :])
                    nc.sync.dma_start(out=wk_t[j], in_=wk[128 * j:128 * (j + 1), :])
                    nc.sync.dma_start(out=wv_t[j], in_=wv[128 * j:128 * (j + 1), :])
                if t == 0 and j == 2:
                    # constants needed from the first RoPE evac onward; emitted
                    # here so they don't delay the first matmul's operands
                    nc.sync.dma_start(out=cos_t, in_=cosT)
                    nc.sync.dma_start(out=sin_t, in_=sinT)
                    nc.sync.dma_start(out=rt_t, in_=rtm)
                    nc.sync.dma_start(out=ones_t, in_=onesd)
                x_t = xpool.tile([128, TG], mm, tag="x")
                nc.sync.dma_start(out=x_t, in_=xT[128 * j:128 * (j + 1), tsl])
                for wi, w_t in enumerate((wq_t, wk_t)):
                    for h in range(LOCAL_H):
                        hsl = slice(128 * h, 128 * (h + 1))
                        nc.tensor.matmul(acc[2 * wi + h], w_t[j][:, hsl],
                                         x_t,
                                         start=(j == 0), stop=(j == ND - 1))
                for m in range(4):
                    msl = slice(128 * m, 128 * (m + 1))
                    nc.tensor.matmul(vacc[m][:, :LOCAL_F], x_t[:, msl],
                                     wv_t[j],
                                     start=(j == 0), stop=(j == ND - 1))
            for wi, res in ((0, qt), (1, kt)):
                for h in range(LOCAL_H):
                    ps = acc[2 * wi + h]
                    # RoPE: res = ps * cos + (Rt.T @ ps) * sin
                    s_t = ropes.tile([HD, TG], mm, tag="s")
                    nc.scalar.copy(s_t, ps)
                    rp = psqk.tile([HD, TG], f32, tag="qk", name="rotp")
                    nc.tensor.matmul(rp, rt_t, s_t, start=True, stop=True)
                    t1 = ropet1.tile([HD, TG], f32, tag="t1")
                    nc.vector.tensor_mul(t1, ps, cos_t[:, csl])
                    t2 = ropet2.tile([HD, TG], f32, tag="t2")
                    nc.vector.tensor_mul(t2, rp, sin_t[:, csl])
                    nc.vector.tensor_add(res[h][:, tsl], t1, t2)
            for m in range(4):
                nc.scalar.copy(v_sb[4 * t + m], vacc[m][:, :LOCAL_F])

    # ---------------- phase 3: attention per (head, batch, query group)
    inb = [dram.tile([N_CORES, HD, TG], mm, name=f"inb{i}")
           for i in range(LOCAL_H)]
    outb = [dram.tile([N_CORES, HD, TG], mm, name=f"outb{i}")
            for i in range(LOCAL_H)]

    wop_scope = tc.tile_pool(name="wop", bufs=10)
    wop = wop_scope.__enter__()
    wo_t = {}

    def load_wo(n, f):
        # per-pass tags: pass1 consumes even f (head-0 features), pass2 odd f;
        # separate slot sets so a pass never holds the other pass's tiles
        tag = "woe" if f % 2 == 0 else "woo"
        w_t = wop.tile([128, TG], mm, tag=tag, name=f"wo{n}_{f}")
        # gpsimd (SWDGE) queue keeps these 16.8MB off the sync queue that
        # carries the latency-critical attnT bounce writes
        nc.gpsimd.dma_start(out=w_t,
                            in_=wo[128 * f:128 * (f + 1),
                                   TG * n:TG * (n + 1)])
        wo_t[(n, f)] = w_t

    # prefetch only as many tiles as the pool has slots; the rest are
    # emitted inside the o_proj loop AFTER the collective triggers, so the
    # in-order gpsimd queue can't cycle (slot release needs o_proj, which
    # needs the AllToAll)
    for n in range(2):
        for f in range(0, ND, 2):
            load_wo(n, f)
    for f in range(1, ND, 2):
        load_wo(0, f)

    with (
        tc.tile_pool(name="maskp", bufs=1) as maskp,
        tc.tile_pool(name="probs", bufs=6) as probs,
        tc.tile_pool(name="recipp", bufs=2) as recipp,
        tc.tile_pool(name="attnp", bufs=4) as attnp,
        tc.tile_pool(name="pssc", bufs=4, space="PSUM") as pssc,
        tc.tile_pool(name="psacc", bufs=2, space="PSUM") as psacc,
    ):
        mask_tiles = {}
        for key, (idx, nb) in mask_idx.items():
            mt = maskp.tile([128, 128 * nb], f32, tag=f"mb{idx}",
                            name=f"mb{idx}")
            nc.sync.dma_start(out=mt, in_=maskc[idx][:, :128 * nb])
            mask_tiles[key] = mt
        LOOKAHEAD = 2  # scores/exp emitted ahead of SUM/PV: PE is in-order,
        # so without lookahead every chunk would stall on the ACT exp latency
        for h in range(LOCAL_H):
            for b, g in [(b, g) for b in range(B) for g in range(4)]:
                if True:
                    chunks = [(j, plan[(g, j)]) for j in range(NB)
                              if (g, j) in plan]
                    qsl = slice(2048 * b + TG * g, 2048 * b + TG * (g + 1))
                    sum_ps = psacc.tile([128, TG], f32, tag="sum")
                    pv_ps = psacc.tile([HD, TG], f32, tag="pv")
                    first = chunks[0][0]
                    last = chunks[-1][0]

                    def emit_scores(idx, h=h, b=b, g=g, chunks=chunks,
                                    qsl=qsl):
                        j, (comp, a0, nb) = chunks[idx]
                        co = 128 * comp
                        ksl = slice(2048 * b + 128 * j,
                                    2048 * b + 128 * (j + 1))
                        sc = pssc.tile([128, TG], f32, tag="sc", name="sc")
                        nc.tensor.matmul(sc[:, co:], kt[h][:, ksl],
                                         qt[h][:, qsl][:, co:],
                                         start=True, stop=True)
                        if nb:
                            mt = mask_tiles[(g, j)]
                            q0 = 128 * a0
                            nc.vector.tensor_add(
                                sc[:, q0:q0 + 128 * nb],
                                sc[:, q0:q0 + 128 * nb], mt)
                        pt = probs.tile([128, TG], mm, tag="p", name="pt")
                        nc.scalar.activation(pt[:, co:], sc[:, co:], Exp,
                                             scale=SCALE)
                        return j, co, pt

                    staged = [emit_scores(i)
                              for i in range(min(LOOKAHEAD, len(chunks)))]
                    for idx in range(len(chunks)):
                        if idx + LOOKAHEAD < len(chunks):
                            staged.append(emit_scores(idx + LOOKAHEAD))
                        j, co, pt = staged.pop(0)
                        nc.tensor.matmul(sum_ps[:, co:], ones_t,
                                         pt[:, co:],
                                         start=(j == first), stop=(j == last))
                        kb = 16 * b + j  # flat token block of this key chunk
                        nc.tensor.matmul(pv_ps[:, co:],
                                         v_sb[kb][:, 128 * h:128 * (h + 1)],
                                         pt[:, co:],
                                         start=(j == first), stop=(j == last))
                    rec = recipp.tile([128, TG], f32, tag="rec")
                    nc.vector.reciprocal(rec, sum_ps)
                    at = attnp.tile([HD, TG], mm, tag="at")
                    nc.vector.tensor_mul(at, pv_ps, rec)
                    s = 4 * b + g  # flat token group = destination rank
                    nc.sync.dma_start(out=inb[h][s], in_=at)
            # AllToAll for this head (head-sharded -> token-sharded);
            # h=0's collective overlaps h=1's attention compute
            nc.gpsimd.collective_compute(
                "AllToAll", mybir.AluOpType.bypass,
                replica_groups=[list(range(N_CORES))],
                ins=[inb[h].opt()], outs=[outb[h].opt()],
            )


    # ---------------- phase 5: output projection for my 512-token slice
    with (
        tc.tile_pool(name="afull", bufs=D // 128) as afull,
        tc.tile_pool(name="outp", bufs=3) as outp,
        tc.tile_pool(name="psop", bufs=3, space="PSUM") as psop,
    ):
        af = [None] * (D // 128)
        for f in ([f for f in range(D // 128) if f % LOCAL_H == 0]
                  + [f for f in range(D // 128) if f % LOCAL_H != 0]):
            a_t = afull.tile([128, TG], mm, tag="af", name=f"af{f}")
            nc.sync.dma_start(out=a_t, in_=outb[f % LOCAL_H][f // LOCAL_H])
            af[f] = a_t
        # pass 1: head-0 feature chunks only -- these land with the first
        # AllToAll, so this entire pass overlaps the second collective.
        # Partial sums are stashed in the dead qt tiles (attention is done
        # with them by now).
        evens = [f for f in range(ND) if f % LOCAL_H == 0]
        odds = [f for f in range(ND) if f % LOCAL_H != 0]
        for n in range(4):
            for f in evens:
                if (n, f) not in wo_t:
                    load_wo(n, f)
            for m in range(4):
                p = 4 * n + m
                ps = psop.tile([128, TG], f32, tag="op", name="op1")
                for i, f in enumerate(evens):
                    nc.tensor.matmul(ps, af[f][:, 128 * m:128 * (m + 1)],
                                     wo_t[(n, f)],
                                     start=(i == 0), stop=(i == len(evens) - 1))
                nc.scalar.copy(qt[p // 8][:, TG * (p % 8):TG * (p % 8 + 1)],
                               ps)
        # pass 2: head-1 feature chunks + the stashed partial
        for n in range(4):
            nsl = slice(TG * n, TG * (n + 1))
            for f in odds:
                load_wo(n, f)
            for m in range(4):
                p = 4 * n + m
                ps = psop.tile([128, TG], f32, tag="op", name="op2")
                for i, f in enumerate(odds):
                    nc.tensor.matmul(ps, af[f][:, 128 * m:128 * (m + 1)],
                                     wo_t[(n, f)],
                                     start=(i == 0), stop=(i == len(odds) - 1))
                o_t = outp.tile([128, TG], f32, tag="o")
                nc.vector.tensor_add(
                    o_t, ps,
                    qt[p // 8][:, TG * (p % 8):TG * (p % 8 + 1)])
                nc.sync.dma_start(out=out[128 * m:128 * (m + 1), nsl], in_=o_t)
    wop_scope.__exit__(None, None, None)
    vres_scope.__exit__(None, None, None)
    qkv_scope.__exit__(None, None, None)


# ---------------------------------------------------------------- entry point
def kernel(x, mask, Wq, Wk, Wv, Wo):
    global last_exec_time_ns
    from concourse.bass_utils import run_bass_kernel_spmd

    x = np.asarray(x, dtype=np.float32)
    mask2d = np.ascontiguousarray(np.asarray(mask, dtype=np.float32)[0, 0])
    Wq = np.asarray(Wq, dtype=np.float32)
    Wk = np.asarray(Wk, dtype=np.float32)
    Wv = np.asarray(Wv, dtype=np.float32)
    Wo = np.ascontiguousarray(np.asarray(Wo, dtype=np.float32))

    # ---- host-side prep
    cls = _classify_mask(mask2d)
    plan = _build_plan(cls)
    maskT_s = None
    mask_idx = {}
    strips = []
    for (g, j), (comp, a0, nb) in sorted(plan.items()):
        if nb == 0:
            continue
        if maskT_s is None:
            maskT_s = np.ascontiguousarray(mask2d.T) * math.sqrt(HD)
        q0 = 512 * g + 128 * a0
        strip = np.zeros((128, 512), dtype=np.float32)
        strip[:, :128 * nb] = maskT_s[128 * j:128 * (j + 1),
                                      q0:q0 + 128 * nb]
        strips.append(strip)
        mask_idx[(g, j)] = (len(strips) - 1, nb)
    maskc = (np.stack(strips) if strips
             else np.zeros((1, 128, 512), dtype=np.float32))

    xTf = np.ascontiguousarray(x.reshape(TOKS, D).T)
    cosT, sinT = _rope_tables()
    rtm = _rot_matrix()

    prec = os.environ.get("KERNEL_PREC", "f32r")
    key = (_plan_key(plan), prec)
    if key not in _NC_CACHE:
        _NC_CACHE[key] = _build(plan, len(strips), mask_idx, prec)
    nc = _NC_CACHE[key]
    ones = np.ones((128, 128), dtype=np.float32)

    in_maps = []
    for c in range(N_CORES):
        fsl = slice(LOCAL_F * c, LOCAL_F * (c + 1))
        in_maps.append({
            "xT": xTf,
            "wq": np.ascontiguousarray(Wq[:, fsl]),
            "wk": np.ascontiguousarray(Wk[:, fsl]),
            "wv": np.ascontiguousarray(Wv[:, fsl]),
            "wo": Wo,
            "maskc": maskc,
            "cosT": cosT,
            "sinT": sinT,
            "rtm": rtm,
            "onesd": ones,
        })

    trace = bool(os.environ.get("KERNEL_TRACE"))
    err = None
    for _ in range(3):
        try:
            res = run_bass_kernel_spmd(nc, in_maps,
                                       core_ids=list(range(N_CORES)),
                                       trace=trace)
            break
        except Exception as e:  # axon transport can be flaky; retry
            err = e
    else:
        raise err

    last_exec_time_ns = res.exec_time_ns
    out_flat = np.concatenate([res.results[c]["out"] for c in range(N_CORES)],
                              axis=0)
    return out_flat.reshape(B, S, D)


# revision 21
# speedup vs baseline: 1.2458x; 1.0328x over previous
"""Trainium2 Bass kernel for HF-style causal self-attention (B=2, S=2048, D=2048,
H=16, head_dim=128), tensor-parallel over heads across 8 NeuronCores.

Sharding: core c computes heads {2c, 2c+1} for both batches (column-sharded
Wq/Wk/Wv). After per-head attention, an 8-rank AllToAll redistributes the
per-head context from head-sharding to token-sharding, and each core runs the
output projection (full Wo) for its 512-token slice. The host concatenates the
8 token slices.

Per-core dataflow (all matmuls run as float32r = full-fp32 at 1 cycle/row):
  QK phase : Q^T,K^T [hd=128, 4096 tok] = Wq/Wk-slice.T @ x, RoPE applied via a
             128x128 rotation matmul + DVE combine with cos/sin tables.
  V phase  : V [tok, 256] natural layout (lhsT = x^T tile).
  Attention: per (head, batch, 512-query-group): scores^T [k,q] chunks on PE,
             causal-trimmed by a runtime classification of the mask;
             exp on ACT (scale=1/sqrt(hd), no max-subtraction -- inputs are
             unit-variance so scores are O(5)); row sums via an all-ones
             matmul accumulated alongside P@V; normalize during PSUM
             evacuation with DVE reciprocal+multiply.
  AllToAll : [8, 256, 512] shards (token groups) -> full-feature 512-token slice.
  O proj   : out[512, 2048] = attn.T @ Wo, streamed Wo tiles.
"""

import math
import os

import numpy as np

# ---------------------------------------------------------------- constants
B, S, D = 2, 2048, 2048
H, HD = 16, 128
N_CORES = 8
LOCAL_H = H // N_CORES  # 2 heads per core
LOCAL_F = LOCAL_H * HD  # 256 local features
TOKS = B * S  # 4096 flat tokens (batch-major)
TG = 512  # token-group width (matmul moving dim)
NT = TOKS // TG  # 8 token groups
NB = S // 128  # 16 key blocks per batch
QB = TG // 128  # 4 query blocks per group
ROPE_BASE = 10000.0
SCALE = 1.0 / math.sqrt(HD)
SKIP_THRESH = -1e8  # mask block entirely masked if all values below this

_NC_CACHE: dict = {}
last_exec_time_ns = None


# ---------------------------------------------------------------- host prep
def _rope_tables():
    inv_freq = 1.0 / (ROPE_BASE ** (np.arange(0, HD, 2, dtype=np.float64) / HD))
    t = np.arange(S, dtype=np.float64)
    freqs = np.outer(t, inv_freq)  # [S, HD/2]
    emb = np.concatenate([freqs, freqs], axis=-1)  # [S, HD]
    cos = np.cos(emb).astype(np.float32).T  # [HD, S]
    sin = np.sin(emb).astype(np.float32).T
    return np.ascontiguousarray(cos), np.ascontiguousarray(sin)


def _rot_matrix():
    # rot = Rt.T @ q  where rot[d] = -q[d+64] (d<64), rot[d] = q[d-64] (d>=64)
    rt = np.zeros((HD, HD), dtype=np.float32)
    half = HD // 2
    for m in range(half):
        rt[m + half, m] = -1.0
    for m in range(half, HD):
        rt[m - half, m] = 1.0
    return rt


def _classify_mask(mask2d):
    """Per 128x128 block of mask[q, k]: 0=all-zero, 1=needs add, 2=fully masked."""
    nq, nk = S // 128, S // 128
    blocks = mask2d.reshape(nq, 128, nk, 128)
    mx = blocks.max(axis=(1, 3))
    mn = blocks.min(axis=(1, 3))
    cls = np.ones((nq, nk), dtype=np.int8)
    cls[(mx == 0.0) & (mn == 0.0)] = 0
    cls[mx < SKIP_THRESH] = 2
    return cls


def _build_plan(cls):
    """For each (query group g, key block j): None if skipped, else
    (comp_start_lqb, add_start_lqb, add_nblocks). The add range spans the
    first to last local query block needing a mask add (interior all-zero
    blocks contribute zeros, so one combined add is exact)."""
    plan = {}
    for g in range(4):
        for j in range(NB):
            lcls = [cls[4 * g + l, j] for l in range(QB)]
            if all(c == 2 for c in lcls):
                continue
            comp = min(l for l in range(QB) if lcls[l] != 2)
            comp = min(comp, 2)  # keep moving dim >= 256 for float32r speed
            adds = [l for l in range(comp, QB) if lcls[l] != 0]
            if adds:
                plan[(g, j)] = (comp, adds[0], adds[-1] - adds[0] + 1)
            else:
                plan[(g, j)] = (comp, 0, 0)
    return plan


def _plan_key(plan):
    return tuple(sorted((k, v) for k, v in plan.items()))


# ---------------------------------------------------------------- bass build
def _build(plan, n_mask_blocks, mask_idx, prec="f32r"):
    import concourse.bacc as bacc
    import concourse.mybir as mybir
    import concourse.tile as tile

    f32 = mybir.dt.float32
    # matmul operand dtype: float32r streams 1 col/cycle (vs 4 for float32)
    # at the cost of ~2^-13 input rounding; toggled for accuracy fallback.
    mm = mybir.dt.float32r if prec == "f32r" else f32
    Exp = mybir.ActivationFunctionType.Exp

    nc = bacc.Bacc("TRN2", target_bir_lowering=False, debug=False,
                   num_devices=N_CORES)

    xT = nc.dram_tensor("xT", [D, TOKS], mm, kind="ExternalInput").ap()
    wq = nc.dram_tensor("wq", [D, LOCAL_F], mm, kind="ExternalInput").ap()
    wk = nc.dram_tensor("wk", [D, LOCAL_F], mm, kind="ExternalInput").ap()
    wv = nc.dram_tensor("wv", [D, LOCAL_F], mm, kind="ExternalInput").ap()
    wo = nc.dram_tensor("wo", [D, D], mm, kind="ExternalInput").ap()
    maskc = nc.dram_tensor("maskc", [max(n_mask_blocks, 1), 128, 512], f32,
                           kind="ExternalInput").ap()
    cosT = nc.dram_tensor("cosT", [HD, S], f32, kind="ExternalInput").ap()
    sinT = nc.dram_tensor("sinT", [HD, S], f32, kind="ExternalInput").ap()
    rtm = nc.dram_tensor("rtm", [HD, HD], mm, kind="ExternalInput").ap()
    onesd = nc.dram_tensor("onesd", [128, 128], mm, kind="ExternalInput").ap()
    out = nc.dram_tensor("out", [TG, D], f32, kind="ExternalOutput").ap()

    with tile.TileContext(nc) as tc:
        with (
            tc.tile_pool(name="const", bufs=1) as constp,
            tc.tile_pool(name="dram", bufs=1, space="DRAM") as dram,
        ):
            # ---- constants
            cos_t = constp.tile([HD, S], f32, tag="cos")
            sin_t = constp.tile([HD, S], f32, tag="sin")
            rt_t = constp.tile([HD, HD], mm, tag="rt")
            ones_t = constp.tile([128, 128], mm, tag="ones")

            _attention_body(nc, tc, tile, mybir, mm, plan, mask_idx,
                            cos_t, sin_t, rt_t, ones_t,
                            xT, wq, wk, wv, wo, out, dram,
                            maskc, cosT, sinT, rtm, onesd)

    nc.compile()
    return nc


def _attention_body(nc, tc, tile, mybir, mm, plan, mask_idx,
                    cos_t, sin_t, rt_t, ones_t,
                    xT, wq, wk, wv, wo, out, dram,
                    maskc, cosT, sinT, rtm, onesd):
    f32 = mybir.dt.float32
    Exp = mybir.ActivationFunctionType.Exp
    ND = D // 128  # 16 contraction chunks

    qkv_scope = tc.tile_pool(name="qkres", bufs=LOCAL_H)
    vres_scope = tc.tile_pool(name="vres", bufs=4 * NT)
    qkres = qkv_scope.__enter__()
    vres = vres_scope.__enter__()

    # resident Q^T / K^T per local head [128, TOKS]
    qt = [qkres.tile([HD, TOKS], mm, tag="qt", name=f"qt{i}") for i in range(LOCAL_H)]
    kt = [qkres.tile([HD, TOKS], mm, tag="kt", name=f"kt{i}") for i in range(LOCAL_H)]
    # resident V tiles [128 tok, LOCAL_F] per flat token block
    v_sb = [vres.tile([128, LOCAL_F], mm, tag="v", name=f"v{i}") for i in range(TOKS // 128)]

    # ---------------- phase 1: QKV projections in ONE x^T sweep
    # Per token group: 8 PSUM accumulators (Q x2 heads, K x2 heads, V x4
    # token blocks); each x^T tile feeds 8 matmuls then releases. RoPE
    # rotation matmuls reuse accumulator slots freed by the Q/K evacuation.
    with (
        tc.tile_pool(name="wpool", bufs=3 * ND) as wpool,
        tc.tile_pool(name="xpool", bufs=8) as xpool,
        tc.tile_pool(name="ropes", bufs=3) as ropes,
        tc.tile_pool(name="ropet", bufs=2) as ropet1,
        tc.tile_pool(name="ropeu", bufs=2) as ropet2,
        tc.tile_pool(name="psqk", bufs=8, space="PSUM") as psqk,
    ):
        wq_t = [wpool.tile([128, LOCAL_F], mm, tag="w", name=f"wqt{i}") for i in range(ND)]
        wk_t = [wpool.tile([128, LOCAL_F], mm, tag="w", name=f"wkt{i}") for i in range(ND)]
        wv_t = [wpool.tile([128, LOCAL_F], mm, tag="w", name=f"wvt{i}") for i in range(ND)]

        for t in range(NT):
            tsl = slice(TG * t, TG * (t + 1))
            csl = slice(TG * (t % 4), TG * (t % 4 + 1))  # batch-local cols
            acc = [psqk.tile([HD, TG], f32, tag="qk", name=f"acc{i}")
                   for i in range(2 * LOCAL_H)]
            vacc = [psqk.tile([128, TG], f32, tag="qk", name=f"vacc{i}")
                    for i in range(4)]
            for j in range(ND):
                if t == 0:
                    nc.sync.dma_start(out=wq_t[j], in_=wq[128 * j:128 * (j + 1), :])
                    nc.sync.dma_start(out=wk_t[j], in_=wk[128 * j:128 * (j + 1), :])
                    nc.sync.dma_start(out=wv_t[j], in_=wv[128 * j:128 * (j + 1), :])
                if t == 0 and j == 2:
                    # constants needed from the first RoPE evac onward; emitted
                    # here so they don't delay the first matmul's operands
                    nc.sync.dma_start(out=cos_t, in_=cosT)
                    nc.sync.dma_start(out=sin_t, in_=sinT)
                    nc.sync.dma_start(out=rt_t, in_=rtm)
                    nc.sync.dma_start(out=ones_t, in_=onesd)
                x_t = xpool.tile([128, TG], mm, tag="x")
                nc.sync.dma_start(out=x_t, in_=xT[128 * j:128 * (j + 1), tsl])
                for wi, w_t in enumerate((wq_t, wk_t)):
                    for h in range(LOCAL_H):
                        hsl = slice(128 * h, 128 * (h + 1))
                        nc.tensor.matmul(acc[2 * wi + h], w_t[j][:, hsl],
                                         x_t,
                                         start=(j == 0), stop=(j == ND - 1))
                for m in range(4):
                    msl = slice(128 * m, 128 * (m + 1))
                    nc.tensor.matmul(vacc[m][:, :LOCAL_F], x_t[:, msl],
                                     wv_t[j],
                                     start=(j == 0), stop=(j == ND - 1))
            for wi, res in ((0, qt), (1, kt)):
                for h in range(LOCAL_H):
                    ps = acc[2 * wi + h]
                    # RoPE: res = ps * cos + (Rt.T @ ps) * sin
                    s_t = ropes.tile([HD, TG], mm, tag="s")
                    nc.scalar.copy(s_t, ps)
                    rp = psqk.tile([HD, TG], f32, tag="qk", name="rotp")
                    nc.tensor.matmul(rp, rt_t, s_t, start=True, stop=True)
                    t1 = ropet1.tile([HD, TG], f32, tag="t1")
                    nc.vector.tensor_mul(t1, ps, cos_t[:, csl])
                    t2 = ropet2.tile([HD, TG], f32, tag="t2")
                    nc.vector.tensor_mul(t2, rp, sin_t[:, csl])
                    nc.vector.tensor_add(res[h][:, tsl], t1, t2)
            for m in range(4):
                nc.scalar.copy(v_sb[4 * t + m], vacc[m][:, :LOCAL_F])

    # ---------------- phase 3: attention per (head, batch, query group)
    inb = [dram.tile([N_CORES, HD, TG], mm, name=f"inb{i}")
           for i in range(LOCAL_H)]
    outb = [dram.tile([N_CORES, HD, TG], mm, name=f"outb{i}")
            for i in range(LOCAL_H)]

    wop_scope = tc.tile_pool(name="wop", bufs=20)
    wop = wop_scope.__enter__()
    wo_t = {}

    # Load order = consumption order: evens (o_proj pass 1, head-0 features)
    # for all n-groups, then odds (pass 2). One shared tag, so slots released
    # by pass 1 recycle into odd-tile prefetch while pass 1 still runs.
    _wo_order = ([(n, f) for n in range(4) for f in range(0, ND, 2)]
                 + [(n, f) for n in range(4) for f in range(1, ND, 2)])

    def load_wo(k):
        n, f = _wo_order[k]
        w_t = wop.tile([128, TG], mm, tag="wo", name=f"wo{n}_{f}")
        # gpsimd (SWDGE) queue keeps these 16.8MB off the sync queue that
        # carries the latency-critical attnT bounce writes
        nc.gpsimd.dma_start(out=w_t,
                            in_=wo[128 * f:128 * (f + 1),
                                   TG * n:TG * (n + 1)])
        wo_t[(n, f)] = w_t

    # prefetch only as many tiles as the pool has slots; the rest are
    # emitted inside the o_proj loop AFTER the collective triggers, so the
    # in-order gpsimd queue can't cycle (slot release needs o_proj, which
    # needs the AllToAll)
    for k in range(20):
        load_wo(k)
    _wo_next = [20]

    def load_wo_upto(k):
        while _wo_next[0] <= k:
            load_wo(_wo_next[0])
            _wo_next[0] += 1

    with (
        tc.tile_pool(name="maskp", bufs=1) as maskp,
        tc.tile_pool(name="probs", bufs=6) as probs,
        tc.tile_pool(name="recipp", bufs=2) as recipp,
        tc.tile_pool(name="attnp", bufs=4) as attnp,
        tc.tile_pool(name="pssc", bufs=4, space="PSUM") as pssc,
        tc.tile_pool(name="psacc", bufs=2, space="PSUM") as psacc,
    ):
        mask_tiles = {}
        for key, (idx, nb) in mask_idx.items():
            mt = maskp.tile([128, 128 * nb], f32, tag=f"mb{idx}",
                            name=f"mb{idx}")
            nc.sync.dma_start(out=mt, in_=maskc[idx][:, :128 * nb])
            mask_tiles[key] = mt
        LOOKAHEAD = 3  # scores/exp emitted ahead of SUM/PV: PE is in-order,
        # so without lookahead every chunk would stall on the ACT exp latency
        for h in range(LOCAL_H):
            for b, g in [(b, g) for b in range(B) for g in range(4)]:
                if True:
                    chunks = [(j, plan[(g, j)]) for j in range(NB)
                              if (g, j) in plan]
                    qsl = slice(2048 * b + TG * g, 2048 * b + TG * (g + 1))
                    sum_ps = psacc.tile([128, TG], f32, tag="sum")
                    pv_ps = psacc.tile([HD, TG], f32, tag="pv")
                    first = chunks[0][0]
                    last = chunks[-1][0]

                    def emit_scores(idx, h=h, b=b, g=g, chunks=chunks,
                                    qsl=qsl):
                        j, (comp, a0, nb) = chunks[idx]
                        co = 128 * comp
                        ksl = slice(2048 * b + 128 * j,
                                    2048 * b + 128 * (j + 1))
                        sc = pssc.tile([128, TG], f32, tag="sc", name="sc")
                        nc.tensor.matmul(sc[:, co:], kt[h][:, ksl],
                                         qt[h][:, qsl][:, co:],
                                         start=True, stop=True)
                        if nb:
                            mt = mask_tiles[(g, j)]
                            q0 = 128 * a0
                            nc.vector.tensor_add(
                                sc[:, q0:q0 + 128 * nb],
                                sc[:, q0:q0 + 128 * nb], mt)
                        pt = probs.tile([128, TG], mm, tag="p", name="pt")
                        nc.scalar.activation(pt[:, co:], sc[:, co:], Exp,
                                             scale=SCALE)
                        return j, co, pt

                    staged = [emit_scores(i)
                              for i in range(min(LOOKAHEAD, len(chunks)))]
                    for idx in range(len(chunks)):
                        if idx + LOOKAHEAD < len(chunks):
                            staged.append(emit_scores(idx + LOOKAHEAD))
                        j, co, pt = staged.pop(0)
                        nc.tensor.matmul(sum_ps[:, co:], ones_t,
                                         pt[:, co:],
                                         start=(j == first), stop=(j == last))
                        kb = 16 * b + j  # flat token block of this key chunk
                        nc.tensor.matmul(pv_ps[:, co:],
                                         v_sb[kb][:, 128 * h:128 * (h + 1)],
                                         pt[:, co:],
                                         start=(j == first), stop=(j == last))
                    rec = recipp.tile([128, TG], f32, tag="rec")
                    nc.vector.reciprocal(rec, sum_ps)
                    at = attnp.tile([HD, TG], mm, tag="at")
                    nc.vector.tensor_mul(at, pv_ps, rec)
                    s = 4 * b + g  # flat token group = destination rank
                    nc.sync.dma_start(out=inb[h][s], in_=at)
            # AllToAll for this head (head-sharded -> token-sharded);
            # h=0's collective overlaps h=1's attention compute
            nc.gpsimd.collective_compute(
                "AllToAll", mybir.AluOpType.bypass,
                replica_groups=[list(range(N_CORES))],
                ins=[inb[h].opt()], outs=[outb[h].opt()],
            )


    # ---------------- phase 5: output projection for my 512-token slice
    with (
        tc.tile_pool(name="afull", bufs=D // 128) as afull,
        tc.tile_pool(name="outp", bufs=3) as outp,
        tc.tile_pool(name="psop", bufs=3, space="PSUM") as psop,
    ):
        af = [None] * (D // 128)
        for f in ([f for f in range(D // 128) if f % LOCAL_H == 0]
                  + [f for f in range(D // 128) if f % LOCAL_H != 0]):
            a_t = afull.tile([128, TG], mm, tag="af", name=f"af{f}")
            nc.sync.dma_start(out=a_t, in_=outb[f % LOCAL_H][f // LOCAL_H])
            af[f] = a_t
        # pass 1: head-0 feature chunks only -- these land with the first
        # AllToAll, so this entire pass overlaps the second collective.
        # Partial sums are stashed in the dead qt tiles (attention is done
        # with them by now).
        evens = [f for f in range(ND) if f % LOCAL_H == 0]
        odds = [f for f in range(ND) if f % LOCAL_H != 0]
        for n in range(4):
            # emit the next block of wo loads (cycle-safe: we're past the
            # collective triggers in the gpsimd queue)
            load_wo_upto(8 * (n + 1) + 19)
            for m in range(4):
                p = 4 * n + m
                ps = psop.tile([128, TG], f32, tag="op", name="op1")
                for i, f in enumerate(evens):
                    nc.tensor.matmul(ps, af[f][:, 128 * m:128 * (m + 1)],
                                     wo_t[(n, f)],
                                     start=(i == 0), stop=(i == len(evens) - 1))
                nc.scalar.copy(qt[p // 8][:, TG * (p % 8):TG * (p % 8 + 1)],
                               ps)
        # pass 2: head-1 feature chunks + the stashed partial
        load_wo_upto(63)
        for n in range(4):
            nsl = slice(TG * n, TG * (n + 1))
            for m in range(4):
                p = 4 * n + m
                ps = psop.tile([128, TG], f32, tag="op", name="op2")
                for i, f in enumerate(odds):
                    nc.tensor.matmul(ps, af[f][:, 128 * m:128 * (m + 1)],
                                     wo_t[(n, f)],
                                     start=(i == 0), stop=(i == len(odds) - 1))
                o_t = outp.tile([128, TG], f32, tag="o")
                nc.vector.tensor_add(
                    o_t, ps,
                    qt[p // 8][:, TG * (p % 8):TG * (p % 8 + 1)])
                nc.sync.dma_start(out=out[128 * m:128 * (m + 1), nsl], in_=o_t)
    wop_scope.__exit__(None, None, None)
    vres_scope.__exit__(None, None, None)
    qkv_scope.__exit__(None, None, None)


# ---------------------------------------------------------------- entry point
def kernel(x, mask, Wq, Wk, Wv, Wo):
    global last_exec_time_ns
    from concourse.bass_utils import run_bass_kernel_spmd

    x = np.asarray(x, dtype=np.float32)
    mask2d = np.ascontiguousarray(np.asarray(mask, dtype=np.float32)[0, 0])
    Wq = np.asarray(Wq, dtype=np.float32)
    Wk = np.asarray(Wk, dtype=np.float32)
    Wv = np.asarray(Wv, dtype=np.float32)
    Wo = np.ascontiguousarray(np.asarray(Wo, dtype=np.float32))

    # ---- host-side prep
    cls = _classify_mask(mask2d)
    plan = _build_plan(cls)
    maskT_s = None
    mask_idx = {}
    strips = []
    for (g, j), (comp, a0, nb) in sorted(plan.items()):
        if nb == 0:
            continue
        if maskT_s is None:
            maskT_s = np.ascontiguousarray(mask2d.T) * math.sqrt(HD)
        q0 = 512 * g + 128 * a0
        strip = np.zeros((128, 512), dtype=np.float32)
        strip[:, :128 * nb] = maskT_s[128 * j:128 * (j + 1),
                                      q0:q0 + 128 * nb]
        strips.append(strip)
        mask_idx[(g, j)] = (len(strips) - 1, nb)
    maskc = (np.stack(strips) if strips
             else np.zeros((1, 128, 512), dtype=np.float32))

    xTf = np.ascontiguousarray(x.reshape(TOKS, D).T)
    cosT, sinT = _rope_tables()
    rtm = _rot_matrix()

    prec = os.environ.get("KERNEL_PREC", "f32r")
    key = (_plan_key(plan), prec)
    if key not in _NC_CACHE:
        _NC_CACHE[key] = _build(plan, len(strips), mask_idx, prec)
    nc = _NC_CACHE[key]
    ones = np.ones((128, 128), dtype=np.float32)

    in_maps = []
    for c in range(N_CORES):
        fsl = slice(LOCAL_F * c, LOCAL_F * (c + 1))
        in_maps.append({
            "xT": xTf,
            "wq": np.ascontiguousarray(Wq[:, fsl]),
            "wk": np.ascontiguousarray(Wk[:, fsl]),
            "wv": np.ascontiguousarray(Wv[:, fsl]),
            "wo": Wo,
            "maskc": maskc,
            "cosT": cosT,
            "sinT": sinT,
            "rtm": rtm,
            "onesd": ones,
        })

    trace = bool(os.environ.get("KERNEL_TRACE"))
    err = None
    for _ in range(3):
        try:
            res = run_bass_kernel_spmd(nc, in_maps,
                                       core_ids=list(range(N_CORES)),
                                       trace=trace)
            break
        except Exception as e:  # axon transport can be flaky; retry
            err = e
    else:
        raise err

    last_exec_time_ns = res.exec_time_ns
    out_flat = np.concatenate([res.results[c]["out"] for c in range(N_CORES)],
                              axis=0)
    return out_flat.reshape(B, S, D)
